# revision 26
# baseline (speedup 1.0000x reference)
"""Trainium2 Bass kernel for nn_CausalSelfAttention_16810501996824.

Head-sharded (tensor-parallel) causal self-attention over 8 NeuronCores:
each core owns 2 of the 16 heads end-to-end (QKV projection, RMS norm,
rotary, causal attention with sigmoid gate and lambda-blended V).  The
per-head context vectors are exchanged with an AllToAll (t-slice swap, 8x
less wire than an AllGather of the full activation), after which every
core computes the full 1024-channel output projection for its own
256-column t-slice.  Host only reshapes/concats shards.

Phase-1 (projections) and phase-2 (attention) instruction emission is
interleaved window-by-window so every engine keeps a backlog; softmax
exps are batched over two PSUM banks; rsqrt/reciprocal run on the scalar
engine as Dsqrt (constants folded into weights); the rotary swap is a
DVE stream_shuffle in a pairwise-interleaved head-dim layout.

Self-contained: hardcodes all shapes; builds + compiles the Bass module on
first call and caches the jitted SPMD executable.
"""
import json

import numpy as np

# ---------------------------------------------------------------------------
# Problem constants
# ---------------------------------------------------------------------------
DIM = 1024
N_HEAD = 16
T = 2048
HD = 64                 # head dim
GATE_IN = 12
ROPE_BASE = 10000.0
ATTN_SCALE = 0.1
EPS = 1e-6
N_CORES = 8
HPC = N_HEAD // N_CORES  # heads per core = 2
C = HPC * HD             # channels per core = 128
NT512 = T // 512         # 4 t-windows
NS128 = T // 128         # 16 s-blocks
TSL = T // N_CORES       # 256: per-core output t-columns (2 slices of 128)

# ---------------------------------------------------------------------------
# Workaround: the staged walrus build allows at most 1 sem wait per
# instruction (2 for EventSemaphore); stock Tile piles multiple waits onto
# one instruction. Split extras onto single-wait NoOps at serialization.
# ---------------------------------------------------------------------------
_WAIT_CAP = {"EventSemaphore": 2}


def _split_multi_waits(bir: dict) -> dict:
    for fn in bir.get("functions", []):
        for blk in fn.get("blocks", []):
            out = []
            changed = False
            for inst in blk.get("instructions", []):
                si = inst.get("sync_info") or {}
                waits = si.get("on_wait") or []
                cap = _WAIT_CAP.get(inst.get("opcode"), 1)
                if len(waits) > cap:
                    changed = True
                    for j, w in enumerate(waits[cap:]):
                        out.append({
                            "debug": inst.get("debug", 0),
                            "engine": inst["engine"],
                            "ins": [], "outs": [],
                            "name": f"{inst['name']}-wsplit{j}",
                            "opcode": "NoOp",
                            "sync_info": {"on_update": [], "on_wait": [w]},
                            "text_hint": "wait_split",
                        })
                    si = dict(si)
                    si["on_wait"] = waits[:cap]
                    inst = dict(inst)
                    inst["sync_info"] = si
                out.append(inst)
            if changed:
                blk["instructions"] = out
    return bir


def _install_patches():
    import concourse.bass as bass
    if getattr(bass.Bass, "_wait_split_patched", False):
        return
    orig = bass.Bass.to_json_bytes

    def patched(self, *a, **k):
        return json.dumps(_split_multi_waits(json.loads(orig(self, *a, **k)))).encode()

    bass.Bass.to_json_bytes = patched
    bass.Bass._wait_split_patched = True


# pairwise swap of adjacent partitions within each 32-partition quadrant
_SHUF_MASK = [(i ^ 1) for i in range(32)]


# ---------------------------------------------------------------------------
# Bass module
# ---------------------------------------------------------------------------

def _build_module(repeat=1, phases=4):
    import concourse.bass as bass
    import concourse.mybir as mybir
    import concourse.tile as tile
    from contextlib import ExitStack

    F32 = mybir.dt.float32
    F32R = mybir.dt.float32r
    BF16 = mybir.dt.bfloat16
    AF = mybir.ActivationFunctionType

    nc = bass.Bass()

    xT = nc.declare_dram_parameter("xT", [DIM, T], F32R, isOutput=False)
    wqkvT = nc.declare_dram_parameter("wqkvT", [DIM, 3 * C], F32R, isOutput=False)
    wgT = nc.declare_dram_parameter("wgT", [GATE_IN, HPC], F32R, isOutput=False)
    wprojT = nc.declare_dram_parameter("wprojT", [DIM, DIM], BF16, isOutput=False)
    v1lam = nc.declare_dram_parameter("v1lam", [T, C], F32R, isOutput=False)
    cosd = nc.declare_dram_parameter("cosd", [C, T], F32, isOutput=False)
    sin4d = nc.declare_dram_parameter("sin4d", [C, T], F32, isOutput=False)
    identm = nc.declare_dram_parameter("identm", [128, 128], F32R, isOutput=False)
    blo2m = nc.declare_dram_parameter("blo2m", [128, 33], F32R, isOutput=False)
    m33m = nc.declare_dram_parameter("m33m", [33, 128], F32R, isOutput=False)
    outT = nc.declare_dram_parameter("outT", [DIM, TSL], F32, isOutput=True)

    a2a_in = [nc.dram_tensor(f"a2a_in{i}", [DIM, 128], BF16) for i in range(2)]
    y_a2a = [nc.dram_tensor(f"y_a2a{i}", [DIM, 128], BF16) for i in range(2)]

    with nc.allow_low_precision(reason="f32r/bf16 matmul pipeline"), \
            tile.TileContext(nc) as tc:
      for _rep in range(repeat):
        with ExitStack() as rep_ctx:
            persist = rep_ctx.enter_context(
                tc.tile_pool(name=f"persist{_rep}", bufs=1))
            vaug_pool = rep_ctx.enter_context(
                tc.tile_pool(name=f"vaug{_rep}", bufs=1))
            # ---- persistent SBUF tiles ----
            qt = persist.tile([128, T], F32R)
            kt = persist.tile([128, T], F32R)
            vt = persist.tile([128, T], F32R)
            gtmp2 = persist.tile([1, HPC * T], F32R)
            ident = persist.tile([128, 128], F32R)
            blo2 = persist.tile([128, 33], F32R)
            m33 = persist.tile([33, 128], F32R)
            eps2 = persist.tile([33, 1], F32)
            ones_c = persist.tile([1, HD], F32R)
            wg_sb = persist.tile([GATE_IN, HPC], F32R)
            xts = [persist.tile([128, T], F32R, name=f"xt{d}", tag=f"xt{d}")
                   for d in range(8)]
            wts = [persist.tile([128, 3 * C], F32R, name=f"wt{d}", tag=f"wt{d}")
                   for d in range(8)]
            wp = [persist.tile([128, DIM], BF16, name=f"wp{cc}", tag=f"wp{cc}")
                  for cc in range(8)]
            dmask = [persist.tile([128, 512], BF16, name=f"dm{k}", tag=f"dm{k}")
                     for k in range(4)]
            v_aug = [[vaug_pool.tile([128, HD + 1], BF16, name=f"va{h}_{si}",
                                     tag=f"va{h}_{si}")
                      for si in range(NS128)] for h in range(HPC)]

            # ---- input DMAs (gpsimd queue; xT/wqkv first: critical path) ----
            for d in range(8):
                nc.gpsimd.dma_start(out=xts[d], in_=xT[128 * d:128 * (d + 1), :])
                nc.gpsimd.dma_start(out=wts[d], in_=wqkvT[128 * d:128 * (d + 1), :])
            nc.sync.dma_start(out=wg_sb, in_=wgT[:])
            nc.sync.dma_start(out=ident, in_=identm[:])
            nc.sync.dma_start(out=blo2, in_=blo2m[:])
            nc.sync.dma_start(out=m33, in_=m33m[:])
            nc.vector.memset(eps2, EPS)
            nc.vector.memset(ones_c.bitcast(F32), 1.0)
            for k in range(4):
                nc.vector.memset(dmask[k], 1.0)
                nc.gpsimd.affine_select(
                    out=dmask[k], in_=dmask[k],
                    compare_op=mybir.AluOpType.is_ge,
                    fill=0.0, base=-(128 * k),
                    channel_multiplier=-1, pattern=[[1, 512]])
            # wproj loads late (only needed by phase 4)
            for cc in range(8):
                nc.scalar.dma_start(out=wp[cc], in_=wprojT[128 * cc:128 * (cc + 1), :])

            # ---- PSUM pools (stack order: early-closing pools open last) ----
            poolA = rep_ctx.enter_context(         # 2x [128,1024] = 4 banks
                tc.tile_pool(name=f"sbig{_rep}", bufs=2, space="PSUM"))
            poolC = rep_ctx.enter_context(         # y accumulators: 2 banks
                tc.tile_pool(name=f"y{_rep}", bufs=1, space="PSUM"))

            p1w = rep_ctx.enter_context(tc.tile_pool(name=f"p1w{_rep}", bufs=1))
            p2w = rep_ctx.enter_context(tc.tile_pool(name=f"p2w{_rep}", bufs=1))

            poolD = ExitStack()                    # smalls g/ms/tr/bc: 1 bank
            pD = poolD.enter_context(
                tc.tile_pool(name=f"small{_rep}", bufs=1, space="PSUM"))
            poolB = ExitStack()                    # qkv accum: 1 bank
            pB = poolB.enter_context(
                tc.tile_pool(name=f"qkv{_rep}", bufs=1, space="PSUM"))

            y_ps = [poolC.tile([HD + 1, 512], F32, name=f"yps{h}", tag=f"y{h}")
                    for h in range(HPC)]

            # =============================================================
            # phase-1 window: QKV projections + RMS norm + rotary + gate
            # =============================================================
            def p1_window(tj):
                ts = slice(512 * tj, 512 * (tj + 1))
                cos_sb = p1w.tile([C, 512], F32, tag="cos", bufs=2)
                sin_sb = p1w.tile([C, 512], F32, tag="sin", bufs=2)
                nc.gpsimd.dma_start(out=cos_sb, in_=cosd[:, ts])
                nc.gpsimd.dma_start(out=sin_sb, in_=sin4d[:, ts])

                # --- gate (own small-bank slot; per-head rows at p0) ---
                for h in range(HPC):
                    g_ps = pD.tile([1, 512], F32, tag="sm", name="gps")
                    nc.tensor.matmul(g_ps, wg_sb[:, h:h + 1],
                                     xts[0][0:GATE_IN, ts],
                                     start=True, stop=True)
                    nc.scalar.activation(gtmp2[:, h * T + 512 * tj:
                                                   h * T + 512 * (tj + 1)],
                                         g_ps, AF.Sigmoid)

                # --- q / k projections + RMS + rotary ---
                for name, col0, dst in (("q", 0, qt), ("k", 128, kt)):
                    r_ps = pB.tile([128, 512], F32, tag="acc")
                    for d in range(8):
                        nc.tensor.matmul(r_ps, wts[d][:, col0:col0 + 128],
                                         xts[d][:, ts], start=(d == 0),
                                         stop=(d == 7))
                    sq = p1w.tile([128, 512], F32R, tag=f"{name}sq")
                    nc.scalar.activation(sq, r_ps, AF.Square)
                    ms = pD.tile([33, 512], F32, tag="sm")
                    nc.tensor.matmul(ms, blo2, sq, start=True, stop=True)
                    rt = p1w.tile([33, 512], F32R, tag=f"{name}rt")
                    nc.scalar.activation(rt, ms, AF.Ln, bias=eps2)
                    # exp(-0.5*ln(m+eps)) = (m+eps)^-1/2
                    nc.scalar.activation(rt, rt, AF.Exp, scale=-0.5)
                    bc_ps = pD.tile([128, 512], F32, tag="sm", name=f"{name}bc")
                    nc.tensor.matmul(bc_ps, m33, rt, start=True, stop=True)
                    u = p1w.tile([128, 512], F32, tag=f"{name}u")
                    nc.vector.tensor_mul(u, r_ps, sin_sb)
                    su = p1w.tile([128, 512], F32, tag=f"{name}su")
                    nc.vector.stream_shuffle(su, u, _SHUF_MASK)
                    t1 = p1w.tile([128, 512], F32, tag=f"{name}t1")
                    nc.vector.tensor_mul(t1, r_ps, cos_sb)
                    t12 = p1w.tile([128, 512], F32, tag=f"{name}t12")
                    nc.vector.tensor_add(t12, t1, su)
                    nc.vector.tensor_mul(dst[:, ts], t12, bc_ps)

                # --- v projection + transpose + lambda blend ---
                v_ps = pB.tile([128, 512], F32, tag="acc")
                for d in range(8):
                    nc.tensor.matmul(v_ps, wts[d][:, 256:384], xts[d][:, ts],
                                     start=(d == 0), stop=(d == 7))
                nc.vector.tensor_copy(vt[:, ts], v_ps)
                for k in range(4):
                    si = 4 * tj + k
                    ss = slice(128 * si, 128 * (si + 1))
                    tr_full = pD.tile([128, 512], F32R, tag="sm", name="trs")
                    tr_ps = tr_full[:, 0:128]
                    nc.tensor.transpose(tr_ps, vt[:, ss], ident)
                    vl = p1w.tile([128, C], F32R, tag="vl")
                    nc.gpsimd.dma_start(out=vl, in_=v1lam[ss, :])
                    for h in range(HPC):
                        va = v_aug[h][si]
                        nc.vector.tensor_add(va[:, 0:HD],
                                             tr_ps[:, HD * h:HD * (h + 1)],
                                             vl[:, HD * h:HD * (h + 1)])
                        nc.vector.memset(va[:, HD:HD + 1], 1.0)

            # =============================================================
            # phase-2 window: causal attention for t-window tj
            # =============================================================
            def p2_window(tj, use_gpsimd):
                ts = slice(512 * tj, 512 * (tj + 1))
                nsb = 4 * tj + 4
                # paired-head groups over si
                pend = {0: None, 1: None}
                for si in range(nsb):
                    ss = slice(128 * si, 128 * (si + 1))
                    s_big = poolA.tile([128, 1024], F32, tag="sbig")
                    for h in range(HPC):
                        hs = slice(HD * h, HD * (h + 1))
                        nc.tensor.matmul(s_big[:, 512 * h:512 * (h + 1)],
                                         kt[hs, ss], qt[hs, ts],
                                         start=True, stop=True)
                    p_sb = p2w.tile([128, 1024], BF16, tag="p", bufs=4)
                    nc.scalar.activation(p_sb, s_big, AF.Exp, scale=ATTN_SCALE)
                    ko = si - 4 * tj
                    if ko >= 0:
                        for h in range(HPC):
                            nc.vector.tensor_mul(p_sb[:, 512 * h:512 * (h + 1)],
                                                 p_sb[:, 512 * h:512 * (h + 1)],
                                                 dmask[ko])
                    # lag PV by one group so exp(si+1) overlaps PV(si)
                    for h in range(HPC):
                        if pend[h] is not None:
                            psi, pp = pend[h]
                            nc.tensor.matmul(y_ps[h], v_aug[h][psi],
                                             pp[:, 512 * h:512 * (h + 1)],
                                             start=(psi == 0), stop=False)
                        pend[h] = (si, p_sb)
                for h in range(HPC):
                    psi, pp = pend[h]
                    nc.tensor.matmul(y_ps[h], v_aug[h][psi],
                                     pp[:, 512 * h:512 * (h + 1)],
                                     start=(psi == 0), stop=True)
                # --- per-head epilogue: 1/den, gate, store to a2a input ---
                half = tj // 2
                js = 4 * (tj % 2)
                for h in range(HPC):
                    dsq = p2w.tile([1, 512], F32R, tag="dsq", bufs=2)
                    nc.scalar.activation(dsq, y_ps[h][HD:HD + 1, :], AF.Ln)
                    # exp(-ln(den)) = 1/den
                    nc.scalar.activation(dsq, dsq, AF.Exp, scale=-1.0)
                    nc.vector.tensor_mul(
                        dsq, dsq,
                        gtmp2[:, h * T + 512 * tj:h * T + 512 * (tj + 1)])
                    bc_ps = pD.tile([HD, 512], F32, tag="sm", name="bcp")
                    nc.tensor.matmul(bc_ps, ones_c, dsq,
                                     start=True, stop=True)
                    bc64 = p2w.tile([HD, 512], F32, tag="bc64", bufs=2)
                    nc.vector.tensor_copy(bc64, bc_ps)
                    yft = p2w.tile([HD, 512], BF16, tag="yft", bufs=2)
                    nc.vector.tensor_mul(yft, y_ps[h][0:HD, :], bc64)
                    dst = a2a_in[half].rearrange("(j c) t -> c j t", j=8)
                    nc.scalar.dma_start(
                        out=dst[HD * h:HD * (h + 1), js:js + 4, :],
                        in_=yft.rearrange("c (j t) -> c j t", j=4))

            # =============================================================
            # phase-4 half: output projection for this core's t-slice
            # =============================================================
            def p4_half(hh, o_pool):
                yfc = p2w.tile([128, 8, 128], BF16, tag=f"yfc{hh}", bufs=1)
                nc.gpsimd.dma_start(
                    out=yfc, in_=y_a2a[hh].rearrange("(cc p) t -> p cc t", p=128))
                o_ps = o_pool.tile([128, 1024], F32, tag="o", name="ops")
                for oc in range(8):
                    for cc in range(8):
                        nc.tensor.matmul(o_ps[:, 128 * oc:128 * (oc + 1)],
                                         wp[cc][:, 128 * oc:128 * (oc + 1)],
                                         yfc[:, cc, :],
                                         start=(cc == 0), stop=(cc == 7))
                o_sb = p2w.tile([128, 1024], F32, tag="osb", bufs=1,
                                name="osb")
                nc.scalar.copy(o_sb, o_ps)
                dst = outT.rearrange("(oc p) t -> p oc t", p=128)
                nc.sync.dma_start(
                    out=dst[:, :, 128 * hh:128 * (hh + 1)],
                    in_=o_sb.rearrange("p (oc t) -> p oc t", oc=8))

            def fire_a2a(i):
                nc.gpsimd.collective_compute(
                    "AllToAll", mybir.AluOpType.bypass,
                    ins=[a2a_in[i][:]], outs=[y_a2a[i][:]],
                    replica_groups=[list(range(N_CORES))],
                )

            # ---- interleaved emission ----
            p1_window(0)
            p1_window(1)
            p2_window(0, use_gpsimd=True)
            p1_window(2)
            p2_window(1, use_gpsimd=True)
            p1_window(3)
            poolB.close()
            p2_window(2, use_gpsimd=True)
            if phases >= 3:
                fire_a2a(0)
            p2_window(3, use_gpsimd=False)
            poolD.close()
            if phases >= 3:
                o_pool = rep_ctx.enter_context(
                    tc.tile_pool(name=f"ops{_rep}", bufs=1, space="PSUM"))
                p4_half(0, o_pool)
                fire_a2a(1)
                p4_half(1, o_pool)
            else:
                o_sb = p2w.tile([128, 256], F32, tag="dbg", bufs=1)
                nc.vector.tensor_copy(o_sb[:, 0:128],
                                      qt[:, 0:128])
                nc.vector.tensor_copy(o_sb[:, 128:256], kt[:, 0:128])
                for oc in range(8):
                    nc.sync.dma_start(out=outT[128 * oc:128 * (oc + 1), :],
                                      in_=o_sb)

    return nc


# ---------------------------------------------------------------------------
# Host-side prep + cached runner
# ---------------------------------------------------------------------------

_PERM = np.empty(HD, np.int64)
for _k in range(32):
    _PERM[2 * _k] = _k
    _PERM[2 * _k + 1] = 32 + _k


def _rotary_tables():
    inv_freq = (np.float32(1.0) / np.power(
        np.float32(ROPE_BASE),
        np.arange(0, HD, 2, dtype=np.float32) / np.float32(HD))).astype(np.float32)
    t = np.arange(T, dtype=np.float32)
    freqs = t[:, None] * inv_freq[None, :]          # [T, 32]
    cos = np.cos(freqs).astype(np.float32).T        # [32, T]
    sin = np.sin(freqs).astype(np.float32).T
    cosd = np.empty((C, T), np.float32)
    sin4d = np.empty((C, T), np.float32)
    for h in range(HPC):
        b = HD * h
        for k in range(32):
            cosd[b + 2 * k] = cos[k]
            cosd[b + 2 * k + 1] = cos[k]
            sin4d[b + 2 * k] = -sin[k]
            sin4d[b + 2 * k + 1] = sin[k]
    return cosd, sin4d


def _perm_head_rows(w):
    out = np.empty_like(w)
    for h in range(HPC):
        out[HD * h:HD * (h + 1)] = w[HD * h + _PERM]
    return out


_CACHE = {}


def _get_runner(repeat=1, phases=4):
    key = f"runner{repeat}_{phases}"
    if key in _CACHE:
        return _CACHE[key]
    _install_patches()
    nc = _build_module(repeat, phases)

    import jax
    import concourse.mybir as mybir
    from jax.sharding import Mesh, PartitionSpec
    from jax.experimental.shard_map import shard_map
    from concourse import bass2jax

    bass2jax.install_neuronx_cc_hook()
    partition_name = nc.partition_id_tensor.name if nc.partition_id_tensor else None
    in_names, out_names, out_avals, zero_outs = [], [], [], []
    for alloc in nc.m.functions[0].allocations:
        if not isinstance(alloc, mybir.MemoryLocationSet):
            continue
        name = alloc.memorylocations[0].name
        if alloc.kind == "ExternalInput":
            if name != partition_name:
                in_names.append(name)
        elif alloc.kind == "ExternalOutput":
            shape = tuple(alloc.tensor_shape)
            dtype = mybir.dt.np(alloc.dtype)
            out_names.append(name)
            out_avals.append(jax.core.ShapedArray(shape, dtype))
            zero_outs.append(np.zeros(shape, dtype))
    all_in_names = in_names + out_names
    if partition_name is not None:
        all_in_names.append(partition_name)
    n_params, n_outs = len(in_names), len(out_avals)

    def _body(*args):
        operands = list(args)
        if partition_name is not None:
            operands.append(bass2jax.partition_id_tensor())
        return tuple(bass2jax._bass_exec_p.bind(
            *operands,
            out_avals=tuple(out_avals),
            in_names=tuple(all_in_names),
            out_names=tuple(out_names),
            lowering_input_output_aliases=(),
            sim_require_finite=True, sim_require_nnan=True, nc=nc,
        ))

    devices = jax.devices()[:N_CORES]
    mesh = Mesh(np.asarray(devices), ("core",))
    fn = jax.jit(
        shard_map(_body, mesh=mesh,
                  in_specs=(PartitionSpec("core"),) * (n_params + n_outs),
                  out_specs=(PartitionSpec("core"),) * n_outs,
                  check_rep=False),
        keep_unused=True,
    )
    state = {
        "fn": fn, "in_names": in_names, "out_names": out_names,
        "out_avals": out_avals, "zero_outs": zero_outs, "nc": nc,
    }
    _CACHE[key] = state
    return state


def _prep_inputs(x, v1, Wq, Wk, Wv, Wproj, lamb, Wgate):
    import ml_dtypes
    x = np.asarray(x, np.float32)
    v1 = np.asarray(v1, np.float32)
    lam = np.float32(np.asarray(lamb))
    xT = np.ascontiguousarray(x[0].T)
    cosd, sin4d = _rotary_tables()
    blo2 = np.zeros((128, 33), np.float32)
    for h in range(HPC):
        blo2[HD * h:HD * (h + 1), 32 * h] = 1.0 / HD
    m33 = np.zeros((33, 128), np.float32)
    for h in range(HPC):
        m33[32 * h, HD * h:HD * (h + 1)] = 1.0
    wproj_b = np.ascontiguousarray(
        np.asarray(Wproj, np.float32).T
    ).astype(ml_dtypes.bfloat16)
    in_maps = []
    for r in range(N_CORES):
        rows = slice(C * r, C * (r + 1))
        heads = slice(HPC * r, HPC * (r + 1))
        wq = _perm_head_rows(np.asarray(Wq)[rows])
        wk = _perm_head_rows(np.asarray(Wk)[rows])
        wv = (np.float32(1.0) - lam) * np.asarray(Wv)[rows]
        wqkvT = np.ascontiguousarray(
            np.concatenate([wq.T, wk.T, wv.T], axis=1).astype(np.float32))
        in_maps.append({
            "xT": xT,
            "wqkvT": wqkvT,
            "wgT": np.ascontiguousarray(
                np.asarray(Wgate)[heads].T.astype(np.float32)),
            "wprojT": wproj_b,
            "v1lam": np.ascontiguousarray((lam * v1[0][:, rows]).astype(np.float32)),
            "cosd": cosd,
            "sin4d": sin4d,
            "identm": np.eye(128, dtype=np.float32),
            "blo2m": blo2,
            "m33m": m33,
        })
    return in_maps


def _run(in_maps):
    st = _get_runner()
    concat_in = [
        np.ascontiguousarray(np.concatenate([in_maps[c][n] for c in range(N_CORES)],
                                            axis=0))
        for n in st["in_names"]
    ]
    concat_zeros = [
        np.zeros((N_CORES * z.shape[0], *z.shape[1:]), z.dtype)
        for z in st["zero_outs"]
    ]
    outs = st["fn"](*concat_in, *concat_zeros)
    outs = [np.asarray(o) for o in outs]
    return {n: outs[i].reshape(N_CORES, *st["out_avals"][i].shape)
            for i, n in enumerate(st["out_names"])}


def kernel(x, v1, Wq, Wk, Wv, Wproj, lamb, Wgate):
    in_maps = _prep_inputs(x, v1, Wq, Wk, Wv, Wproj, lamb, Wgate)
    res = _run(in_maps)
    outT = res["outT"]                                     # [cores, DIM, TSL]
    y = np.empty((1, T, DIM), np.float32)
    for r in range(N_CORES):
        y[0, 128 * r:128 * (r + 1), :] = outT[r][:, 0:128].T
        y[0, 1024 + 128 * r:1024 + 128 * (r + 1), :] = outT[r][:, 128:256].T
    return y, np.asarray(v1, np.float32)


# revision 28
# speedup vs baseline: 2.0955x; 2.0955x over previous
"""Trainium2 Bass kernel for nn_CausalSelfAttention_16810501996824.

Head-sharded (tensor-parallel) causal self-attention over 8 NeuronCores:
each core owns 2 of the 16 heads end-to-end (QKV projection, RMS norm,
rotary, causal attention with sigmoid gate and lambda-blended V).  The
per-head context vectors are exchanged with an AllToAll (t-slice swap, 8x
less wire than an AllGather of the full activation), after which every
core computes the full 1024-channel output projection for its own
256-column t-slice.  Host only reshapes/concats shards.

Phase-1 (projections) and phase-2 (attention) instruction emission is
interleaved window-by-window so every engine keeps a backlog; softmax
exps are batched over two PSUM banks; rsqrt/reciprocal run on the scalar
engine as Dsqrt (constants folded into weights); the rotary swap is a
DVE stream_shuffle in a pairwise-interleaved head-dim layout.

Self-contained: hardcodes all shapes; builds + compiles the Bass module on
first call and caches the jitted SPMD executable.
"""
import json

import numpy as np

# ---------------------------------------------------------------------------
# Problem constants
# ---------------------------------------------------------------------------
DIM = 1024
N_HEAD = 16
T = 2048
HD = 64                 # head dim
GATE_IN = 12
ROPE_BASE = 10000.0
ATTN_SCALE = 0.1
EPS = 1e-6
N_CORES = 8
HPC = N_HEAD // N_CORES  # heads per core = 2
C = HPC * HD             # channels per core = 128
NT512 = T // 512         # 4 t-windows
NS128 = T // 128         # 16 s-blocks
TSL = T // N_CORES       # 256: per-core output t-columns (2 slices of 128)

# ---------------------------------------------------------------------------
# Workaround: the staged walrus build allows at most 1 sem wait per
# instruction (2 for EventSemaphore); stock Tile piles multiple waits onto
# one instruction. Split extras onto single-wait NoOps at serialization.
# ---------------------------------------------------------------------------
_WAIT_CAP = {"EventSemaphore": 2}


def _split_multi_waits(bir: dict) -> dict:
    for fn in bir.get("functions", []):
        for blk in fn.get("blocks", []):
            out = []
            changed = False
            for inst in blk.get("instructions", []):
                si = inst.get("sync_info") or {}
                waits = si.get("on_wait") or []
                cap = _WAIT_CAP.get(inst.get("opcode"), 1)
                if len(waits) > cap:
                    changed = True
                    for j, w in enumerate(waits[cap:]):
                        out.append({
                            "debug": inst.get("debug", 0),
                            "engine": inst["engine"],
                            "ins": [], "outs": [],
                            "name": f"{inst['name']}-wsplit{j}",
                            "opcode": "NoOp",
                            "sync_info": {"on_update": [], "on_wait": [w]},
                            "text_hint": "wait_split",
                        })
                    si = dict(si)
                    si["on_wait"] = waits[:cap]
                    inst = dict(inst)
                    inst["sync_info"] = si
                out.append(inst)
            if changed:
                blk["instructions"] = out
    return bir


def _install_patches():
    import concourse.bass as bass
    if getattr(bass.Bass, "_wait_split_patched", False):
        return
    orig = bass.Bass.to_json_bytes

    def patched(self, *a, **k):
        return json.dumps(_split_multi_waits(json.loads(orig(self, *a, **k)))).encode()

    bass.Bass.to_json_bytes = patched
    bass.Bass._wait_split_patched = True


# pairwise swap of adjacent partitions within each 32-partition quadrant
_SHUF_MASK = [(i ^ 1) for i in range(32)]


# ---------------------------------------------------------------------------
# Bass module
# ---------------------------------------------------------------------------

def _build_module(repeat=1, phases=4):
    import concourse.bass as bass
    import concourse.mybir as mybir
    import concourse.tile as tile
    from contextlib import ExitStack

    F32 = mybir.dt.float32
    F32R = mybir.dt.float32r
    BF16 = mybir.dt.bfloat16
    AF = mybir.ActivationFunctionType

    nc = bass.Bass()

    xT = nc.declare_dram_parameter("xT", [DIM, T], BF16, isOutput=False)
    wqkvT = nc.declare_dram_parameter("wqkvT", [DIM, 3 * C], BF16, isOutput=False)
    wgT = nc.declare_dram_parameter("wgT", [GATE_IN, HPC], BF16, isOutput=False)
    wprojT = nc.declare_dram_parameter("wprojT", [DIM, DIM], BF16, isOutput=False)
    v1lam = nc.declare_dram_parameter("v1lam", [T, C], BF16, isOutput=False)
    cosd = nc.declare_dram_parameter("cosd", [C, T], F32, isOutput=False)
    sin4d = nc.declare_dram_parameter("sin4d", [C, T], F32, isOutput=False)
    identm = nc.declare_dram_parameter("identm", [128, 128], F32R, isOutput=False)
    blo2m = nc.declare_dram_parameter("blo2m", [128, 33], F32R, isOutput=False)
    m33m = nc.declare_dram_parameter("m33m", [33, 128], F32R, isOutput=False)
    outT = nc.declare_dram_parameter("outT", [DIM, TSL], F32, isOutput=True)

    a2a_in = [nc.dram_tensor(f"a2a_in{i}", [DIM, 128], BF16) for i in range(2)]
    y_a2a = [nc.dram_tensor(f"y_a2a{i}", [DIM, 128], BF16) for i in range(2)]

    with nc.allow_low_precision(reason="f32r/bf16 matmul pipeline"), \
            tile.TileContext(nc) as tc:
      for _rep in range(repeat):
        with ExitStack() as rep_ctx:
            persist = rep_ctx.enter_context(
                tc.tile_pool(name=f"persist{_rep}", bufs=1))
            vaug_pool = rep_ctx.enter_context(
                tc.tile_pool(name=f"vaug{_rep}", bufs=1))
            # ---- persistent SBUF tiles ----
            qt = persist.tile([128, T], F32R)
            kt = persist.tile([128, T], F32R)
            vt = persist.tile([128, T], F32R)
            gtmp2 = persist.tile([1, HPC * T], F32R)
            ident = persist.tile([128, 128], F32R)
            blo2 = persist.tile([128, 33], F32R)
            m33 = persist.tile([33, 128], F32R)
            eps2 = persist.tile([33, 1], F32)
            ones_c = persist.tile([1, HD], F32R)
            wg_sb = persist.tile([GATE_IN, HPC], BF16)
            xts = [persist.tile([128, T], BF16, name=f"xt{d}", tag=f"xt{d}")
                   for d in range(8)]
            wts = [persist.tile([128, 3 * C], BF16, name=f"wt{d}", tag=f"wt{d}")
                   for d in range(8)]
            wp = [persist.tile([128, DIM], BF16, name=f"wp{cc}", tag=f"wp{cc}")
                  for cc in range(8)]
            dmask = [persist.tile([128, 512], BF16, name=f"dm{k}", tag=f"dm{k}")
                     for k in range(4)]
            v_aug = [[vaug_pool.tile([128, HD + 1], BF16, name=f"va{h}_{si}",
                                     tag=f"va{h}_{si}")
                      for si in range(NS128)] for h in range(HPC)]

            # ---- input DMAs spread across queues; xT/wqkv = critical path ----
            for d in range(8):
                q = (nc.gpsimd, nc.scalar, nc.sync)[d % 3]
                q.dma_start(out=xts[d], in_=xT[128 * d:128 * (d + 1), :])
                q.dma_start(out=wts[d], in_=wqkvT[128 * d:128 * (d + 1), :])
            nc.sync.dma_start(out=wg_sb, in_=wgT[:])
            nc.sync.dma_start(out=ident, in_=identm[:])
            nc.sync.dma_start(out=blo2, in_=blo2m[:])
            nc.sync.dma_start(out=m33, in_=m33m[:])
            nc.vector.memset(eps2, EPS)
            nc.vector.memset(ones_c.bitcast(F32), 1.0)
            for k in range(4):
                nc.vector.memset(dmask[k], 1.0)
                nc.gpsimd.affine_select(
                    out=dmask[k], in_=dmask[k],
                    compare_op=mybir.AluOpType.is_ge,
                    fill=0.0, base=-(128 * k),
                    channel_multiplier=-1, pattern=[[1, 512]])
            # wproj loads late (only needed by phase 4)
            for cc in range(8):
                nc.scalar.dma_start(out=wp[cc], in_=wprojT[128 * cc:128 * (cc + 1), :])

            # ---- PSUM pools (stack order: early-closing pools open last) ----
            poolA = rep_ctx.enter_context(         # 2x [128,1024] = 4 banks
                tc.tile_pool(name=f"sbig{_rep}", bufs=2, space="PSUM"))
            poolC = rep_ctx.enter_context(         # y accumulators: 2 banks
                tc.tile_pool(name=f"y{_rep}", bufs=1, space="PSUM"))

            p1w = rep_ctx.enter_context(tc.tile_pool(name=f"p1w{_rep}", bufs=1))
            p2w = rep_ctx.enter_context(tc.tile_pool(name=f"p2w{_rep}", bufs=1))

            poolD = ExitStack()                    # smalls g/ms/tr/bc: 1 bank
            pD = poolD.enter_context(
                tc.tile_pool(name=f"small{_rep}", bufs=1, space="PSUM"))
            poolB = ExitStack()                    # qkv accum: 1 bank
            pB = poolB.enter_context(
                tc.tile_pool(name=f"qkv{_rep}", bufs=1, space="PSUM"))

            y_ps = [poolC.tile([HD + 1, 512], F32, name=f"yps{h}", tag=f"y{h}")
                    for h in range(HPC)]

            # =============================================================
            # phase-1 window: QKV projections + RMS norm + rotary + gate
            # =============================================================
            def p1_window(tj):
                ts = slice(512 * tj, 512 * (tj + 1))
                cos_sb = p1w.tile([C, 512], F32, tag="cos", bufs=2)
                sin_sb = p1w.tile([C, 512], F32, tag="sin", bufs=2)
                nc.sync.dma_start(out=cos_sb, in_=cosd[:, ts])
                nc.sync.dma_start(out=sin_sb, in_=sin4d[:, ts])

                # --- gate (own small-bank slot; per-head rows at p0) ---
                for h in range(HPC):
                    g_ps = pD.tile([1, 512], F32, tag="sm", name="gps")
                    nc.tensor.matmul(g_ps, wg_sb[:, h:h + 1],
                                     xts[0][0:GATE_IN, ts],
                                     start=True, stop=True)
                    nc.scalar.activation(gtmp2[:, h * T + 512 * tj:
                                                   h * T + 512 * (tj + 1)],
                                         g_ps, AF.Sigmoid)

                # --- q / k projections + RMS + rotary ---
                for name, col0, dst in (("q", 0, qt), ("k", 128, kt)):
                    r_ps = pB.tile([128, 512], F32, tag="acc")
                    for d in range(8):
                        nc.tensor.matmul(r_ps, wts[d][:, col0:col0 + 128],
                                         xts[d][:, ts], start=(d == 0),
                                         stop=(d == 7))
                    sq = p1w.tile([128, 512], F32R, tag=f"{name}sq")
                    nc.scalar.activation(sq, r_ps, AF.Square)
                    ms = pD.tile([33, 512], F32, tag="sm")
                    nc.tensor.matmul(ms, blo2, sq, start=True, stop=True)
                    rt = p1w.tile([33, 512], F32R, tag=f"{name}rt")
                    nc.scalar.activation(rt, ms, AF.Ln, bias=eps2)
                    # exp(-0.5*ln(m+eps)) = (m+eps)^-1/2
                    nc.scalar.activation(rt, rt, AF.Exp, scale=-0.5)
                    bc_ps = pD.tile([128, 512], F32, tag="sm", name=f"{name}bc")
                    nc.tensor.matmul(bc_ps, m33, rt, start=True, stop=True)
                    u = p1w.tile([128, 512], F32, tag=f"{name}u")
                    nc.vector.tensor_mul(u, r_ps, sin_sb)
                    su = p1w.tile([128, 512], F32, tag=f"{name}su")
                    nc.vector.stream_shuffle(su, u, _SHUF_MASK)
                    t1 = p1w.tile([128, 512], F32, tag=f"{name}t1")
                    nc.vector.tensor_mul(t1, r_ps, cos_sb)
                    t12 = p1w.tile([128, 512], F32, tag=f"{name}t12")
                    nc.vector.tensor_add(t12, t1, su)
                    nc.vector.tensor_mul(dst[:, ts], t12, bc_ps)

                # --- v projection + transpose + lambda blend ---
                v_ps = pB.tile([128, 512], F32, tag="acc")
                for d in range(8):
                    nc.tensor.matmul(v_ps, wts[d][:, 256:384], xts[d][:, ts],
                                     start=(d == 0), stop=(d == 7))
                nc.vector.tensor_copy(vt[:, ts], v_ps)
                for k in range(4):
                    si = 4 * tj + k
                    ss = slice(128 * si, 128 * (si + 1))
                    tr_full = pD.tile([128, 512], F32R, tag="sm", name="trs")
                    tr_ps = tr_full[:, 0:128]
                    nc.tensor.transpose(tr_ps, vt[:, ss], ident)
                    vl = p1w.tile([128, C], BF16, tag="vl")
                    nc.sync.dma_start(out=vl, in_=v1lam[ss, :])
                    for h in range(HPC):
                        va = v_aug[h][si]
                        nc.vector.tensor_add(va[:, 0:HD],
                                             tr_ps[:, HD * h:HD * (h + 1)],
                                             vl[:, HD * h:HD * (h + 1)])
                        nc.vector.memset(va[:, HD:HD + 1], 1.0)

            # =============================================================
            # phase-2 window: causal attention for t-window tj
            # =============================================================
            def p2_window(tj, use_gpsimd):
                ts = slice(512 * tj, 512 * (tj + 1))
                nsb = 4 * tj + 4
                # paired-head groups over si
                pend = {0: None, 1: None}
                for si in range(nsb):
                    ss = slice(128 * si, 128 * (si + 1))
                    s_big = poolA.tile([128, 1024], F32, tag="sbig")
                    for h in range(HPC):
                        hs = slice(HD * h, HD * (h + 1))
                        nc.tensor.matmul(s_big[:, 512 * h:512 * (h + 1)],
                                         kt[hs, ss], qt[hs, ts],
                                         start=True, stop=True)
                    p_sb = p2w.tile([128, 1024], BF16, tag="p", bufs=4)
                    nc.scalar.activation(p_sb, s_big, AF.Exp, scale=ATTN_SCALE)
                    ko = si - 4 * tj
                    if ko >= 0:
                        for h in range(HPC):
                            nc.vector.tensor_mul(p_sb[:, 512 * h:512 * (h + 1)],
                                                 p_sb[:, 512 * h:512 * (h + 1)],
                                                 dmask[ko])
                    # lag PV by one group so exp(si+1) overlaps PV(si)
                    for h in range(HPC):
                        if pend[h] is not None:
                            psi, pp = pend[h]
                            nc.tensor.matmul(y_ps[h], v_aug[h][psi],
                                             pp[:, 512 * h:512 * (h + 1)],
                                             start=(psi == 0), stop=False)
                        pend[h] = (si, p_sb)
                for h in range(HPC):
                    psi, pp = pend[h]
                    nc.tensor.matmul(y_ps[h], v_aug[h][psi],
                                     pp[:, 512 * h:512 * (h + 1)],
                                     start=(psi == 0), stop=True)
                # --- per-head epilogue: 1/den, gate, store to a2a input ---
                half = tj // 2
                js = 4 * (tj % 2)
                for h in range(HPC):
                    dsq = p2w.tile([1, 512], F32R, tag="dsq", bufs=2)
                    nc.scalar.activation(dsq, y_ps[h][HD:HD + 1, :], AF.Ln)
                    # exp(-ln(den)) = 1/den
                    nc.scalar.activation(dsq, dsq, AF.Exp, scale=-1.0)
                    nc.vector.tensor_mul(
                        dsq, dsq,
                        gtmp2[:, h * T + 512 * tj:h * T + 512 * (tj + 1)])
                    bc_ps = pD.tile([HD, 512], F32, tag="sm", name="bcp")
                    nc.tensor.matmul(bc_ps, ones_c, dsq,
                                     start=True, stop=True)
                    bc64 = p2w.tile([HD, 512], F32, tag="bc64", bufs=2)
                    nc.vector.tensor_copy(bc64, bc_ps)
                    yft = p2w.tile([HD, 512], BF16, tag="yft", bufs=2)
                    nc.vector.tensor_mul(yft, y_ps[h][0:HD, :], bc64)
                    dst = a2a_in[half].rearrange("(j c) t -> c j t", j=8)
                    nc.scalar.dma_start(
                        out=dst[HD * h:HD * (h + 1), js:js + 4, :],
                        in_=yft.rearrange("c (j t) -> c j t", j=4))

            # =============================================================
            # phase-4 half: output projection for this core's t-slice
            # =============================================================
            def p4_half(hh, o_pool):
                yfc = p2w.tile([128, 8, 128], BF16, tag=f"yfc{hh}", bufs=1)
                nc.gpsimd.dma_start(
                    out=yfc, in_=y_a2a[hh].rearrange("(cc p) t -> p cc t", p=128))
                o_ps = o_pool.tile([128, 1024], F32, tag="o", name="ops")
                for oc in range(8):
                    for cc in range(8):
                        nc.tensor.matmul(o_ps[:, 128 * oc:128 * (oc + 1)],
                                         wp[cc][:, 128 * oc:128 * (oc + 1)],
                                         yfc[:, cc, :],
                                         start=(cc == 0), stop=(cc == 7))
                o_sb = p2w.tile([128, 1024], F32, tag="osb", bufs=1,
                                name="osb")
                nc.scalar.copy(o_sb, o_ps)
                dst = outT.rearrange("(oc p) t -> p oc t", p=128)
                nc.sync.dma_start(
                    out=dst[:, :, 128 * hh:128 * (hh + 1)],
                    in_=o_sb.rearrange("p (oc t) -> p oc t", oc=8))

            def fire_a2a(i):
                nc.gpsimd.collective_compute(
                    "AllToAll", mybir.AluOpType.bypass,
                    ins=[a2a_in[i][:]], outs=[y_a2a[i][:]],
                    replica_groups=[list(range(N_CORES))],
                )

            # ---- interleaved emission ----
            p1_window(0)
            p1_window(1)
            p2_window(0, use_gpsimd=True)
            p1_window(2)
            p2_window(1, use_gpsimd=True)
            if phases >= 3:
                fire_a2a(0)
            p1_window(3)
            poolB.close()
            p2_window(2, use_gpsimd=True)
            p2_window(3, use_gpsimd=False)
            poolD.close()
            if phases >= 3:
                o_pool = rep_ctx.enter_context(
                    tc.tile_pool(name=f"ops{_rep}", bufs=1, space="PSUM"))
                p4_half(0, o_pool)
                fire_a2a(1)
                p4_half(1, o_pool)
            else:
                o_sb = p2w.tile([128, 256], F32, tag="dbg", bufs=1)
                nc.vector.tensor_copy(o_sb[:, 0:128],
                                      qt[:, 0:128])
                nc.vector.tensor_copy(o_sb[:, 128:256], kt[:, 0:128])
                for oc in range(8):
                    nc.sync.dma_start(out=outT[128 * oc:128 * (oc + 1), :],
                                      in_=o_sb)

    return nc


# ---------------------------------------------------------------------------
# Host-side prep + cached runner
# ---------------------------------------------------------------------------

_PERM = np.empty(HD, np.int64)
for _k in range(32):
    _PERM[2 * _k] = _k
    _PERM[2 * _k + 1] = 32 + _k


def _rotary_tables():
    inv_freq = (np.float32(1.0) / np.power(
        np.float32(ROPE_BASE),
        np.arange(0, HD, 2, dtype=np.float32) / np.float32(HD))).astype(np.float32)
    t = np.arange(T, dtype=np.float32)
    freqs = t[:, None] * inv_freq[None, :]          # [T, 32]
    cos = np.cos(freqs).astype(np.float32).T        # [32, T]
    sin = np.sin(freqs).astype(np.float32).T
    cosd = np.empty((C, T), np.float32)
    sin4d = np.empty((C, T), np.float32)
    for h in range(HPC):
        b = HD * h
        for k in range(32):
            cosd[b + 2 * k] = cos[k]
            cosd[b + 2 * k + 1] = cos[k]
            sin4d[b + 2 * k] = -sin[k]
            sin4d[b + 2 * k + 1] = sin[k]
    return cosd, sin4d


def _perm_head_rows(w):
    out = np.empty_like(w)
    for h in range(HPC):
        out[HD * h:HD * (h + 1)] = w[HD * h + _PERM]
    return out


_CACHE = {}


def _get_runner(repeat=1, phases=4):
    key = f"runner{repeat}_{phases}"
    if key in _CACHE:
        return _CACHE[key]
    _install_patches()
    nc = _build_module(repeat, phases)

    import jax
    import concourse.mybir as mybir
    from jax.sharding import Mesh, PartitionSpec
    from jax.experimental.shard_map import shard_map
    from concourse import bass2jax

    bass2jax.install_neuronx_cc_hook()
    partition_name = nc.partition_id_tensor.name if nc.partition_id_tensor else None
    in_names, out_names, out_avals, zero_outs = [], [], [], []
    for alloc in nc.m.functions[0].allocations:
        if not isinstance(alloc, mybir.MemoryLocationSet):
            continue
        name = alloc.memorylocations[0].name
        if alloc.kind == "ExternalInput":
            if name != partition_name:
                in_names.append(name)
        elif alloc.kind == "ExternalOutput":
            shape = tuple(alloc.tensor_shape)
            dtype = mybir.dt.np(alloc.dtype)
            out_names.append(name)
            out_avals.append(jax.core.ShapedArray(shape, dtype))
            zero_outs.append(np.zeros(shape, dtype))
    all_in_names = in_names + out_names
    if partition_name is not None:
        all_in_names.append(partition_name)
    n_params, n_outs = len(in_names), len(out_avals)

    def _body(*args):
        operands = list(args)
        if partition_name is not None:
            operands.append(bass2jax.partition_id_tensor())
        return tuple(bass2jax._bass_exec_p.bind(
            *operands,
            out_avals=tuple(out_avals),
            in_names=tuple(all_in_names),
            out_names=tuple(out_names),
            lowering_input_output_aliases=(),
            sim_require_finite=True, sim_require_nnan=True, nc=nc,
        ))

    devices = jax.devices()[:N_CORES]
    mesh = Mesh(np.asarray(devices), ("core",))
    fn = jax.jit(
        shard_map(_body, mesh=mesh,
                  in_specs=(PartitionSpec("core"),) * (n_params + n_outs),
                  out_specs=(PartitionSpec("core"),) * n_outs,
                  check_rep=False),
        keep_unused=True,
    )
    state = {
        "fn": fn, "in_names": in_names, "out_names": out_names,
        "out_avals": out_avals, "zero_outs": zero_outs, "nc": nc,
    }
    _CACHE[key] = state
    return state


def _prep_inputs(x, v1, Wq, Wk, Wv, Wproj, lamb, Wgate):
    import ml_dtypes
    x = np.asarray(x, np.float32)
    v1 = np.asarray(v1, np.float32)
    lam = np.float32(np.asarray(lamb))
    xT_b = np.ascontiguousarray(x[0].T).astype(ml_dtypes.bfloat16)
    cosd, sin4d = _rotary_tables()
    blo2 = np.zeros((128, 33), np.float32)
    for h in range(HPC):
        blo2[HD * h:HD * (h + 1), 32 * h] = 1.0 / HD
    m33 = np.zeros((33, 128), np.float32)
    for h in range(HPC):
        m33[32 * h, HD * h:HD * (h + 1)] = 1.0
    wproj_b = np.ascontiguousarray(
        np.asarray(Wproj, np.float32).T
    ).astype(ml_dtypes.bfloat16)
    in_maps = []
    for r in range(N_CORES):
        rows = slice(C * r, C * (r + 1))
        heads = slice(HPC * r, HPC * (r + 1))
        wq = _perm_head_rows(np.asarray(Wq)[rows])
        wk = _perm_head_rows(np.asarray(Wk)[rows])
        wv = (np.float32(1.0) - lam) * np.asarray(Wv)[rows]
        wqkvT = np.ascontiguousarray(
            np.concatenate([wq.T, wk.T, wv.T], axis=1)).astype(ml_dtypes.bfloat16)
        in_maps.append({
            "xT": xT_b,
            "wqkvT": wqkvT,
            "wgT": np.ascontiguousarray(
                np.asarray(Wgate)[heads].T.astype(np.float32)
            ).astype(ml_dtypes.bfloat16),
            "wprojT": wproj_b,
            "v1lam": np.ascontiguousarray(
                lam * v1[0][:, rows]).astype(ml_dtypes.bfloat16),
            "cosd": cosd,
            "sin4d": sin4d,
            "identm": np.eye(128, dtype=np.float32),
            "blo2m": blo2,
            "m33m": m33,
        })
    return in_maps


def _run(in_maps):
    st = _get_runner()
    concat_in = [
        np.ascontiguousarray(np.concatenate([in_maps[c][n] for c in range(N_CORES)],
                                            axis=0))
        for n in st["in_names"]
    ]
    concat_zeros = [
        np.zeros((N_CORES * z.shape[0], *z.shape[1:]), z.dtype)
        for z in st["zero_outs"]
    ]
    outs = st["fn"](*concat_in, *concat_zeros)
    outs = [np.asarray(o) for o in outs]
    return {n: outs[i].reshape(N_CORES, *st["out_avals"][i].shape)
            for i, n in enumerate(st["out_names"])}


def kernel(x, v1, Wq, Wk, Wv, Wproj, lamb, Wgate):
    in_maps = _prep_inputs(x, v1, Wq, Wk, Wv, Wproj, lamb, Wgate)
    res = _run(in_maps)
    outT = res["outT"]                                     # [cores, DIM, TSL]
    y = np.empty((1, T, DIM), np.float32)
    for r in range(N_CORES):
        y[0, 128 * r:128 * (r + 1), :] = outT[r][:, 0:128].T
        y[0, 1024 + 128 * r:1024 + 128 * (r + 1), :] = outT[r][:, 128:256].T
    return y, np.asarray(v1, np.float32)
